# revision 1
# baseline (speedup 1.0000x reference)
"""Diffeomorphic image warp on Trainium2 (8 NeuronCores, batch-data-parallel).

out = bilinear_warp(img, dx, dy); dx/dy are smooth random fields from
100x100 mode coefficients via sin bases (input-independent bases baked
as constants). Per core: 12 channel-images (4 batches x 3 channels).

On-device pipeline per core:
  PE:     dx/dy = S @ (c * E') @ S^T  (negated+scaled E' folded in)
  DVE:    index + weight maps (f32), exact floor/ceil via mod/is_gt
  DMA:    wrapped-index reorder via DRAM round-trip; weight replication
  GPSIMD: 4-tap ap_gather with block-shared wrapped int16 indices
  DVE:    bilinear combine (4 mult + 3 add), DMA out
"""
import math
import sys
from contextlib import ExitStack

import numpy as np

sys.path.insert(0, "/opt/trn_rl_repo")

N = 512
M = 100
NCORES = 8
CPC = 12
BAND_ROWS = 44
BAND = BAND_ROWS * N
SLICE_ROWS = 2
SLICE = SLICE_ROWS * N
PASSES = 2
GROUP_ROWS = 32
SLICES = GROUP_ROWS // SLICE_ROWS
NSL = PASSES * SLICES


def _r0(b, g):
    return min(max(64 * b + 32 * g - 5, 0), N - BAND_ROWS)


def _constants():
    log_cut = math.log(M + 1e-06)
    T1 = 1.0 / (math.pi * N ** 2 * log_cut)
    T2 = max(T1, 4.0 / (math.pi ** 3 * M ** 2 * log_cut))
    T = 0.5 * (T1 + T2)
    scale = math.sqrt(T) * N

    x = np.linspace(0.0, 1.0, N, dtype=np.float64)
    k = np.arange(1, M + 1, dtype=np.float64)
    i, j = np.meshgrid(k, k, indexing="ij")
    r = np.sqrt(i ** 2 + j ** 2)
    e = (r < M + 0.5).astype(np.float64) / r
    s = np.sin(np.pi * x[:, None] * k[None, :])
    S_T = np.ascontiguousarray(s.T).astype(np.float32)
    E_NEG = (-(e * scale)).astype(np.float32)

    x_ramp = np.tile(np.arange(N, dtype=np.float32), (128, 1))
    y_scal = np.zeros((128, 4), dtype=np.float32)
    r0neg512 = np.zeros((128, 4), dtype=np.float32)
    for p in range(128):
        for c in range(4):
            y = 128 * c + p
            y_scal[p, c] = y
            b, g = y // 64, (y // 32) % 2
            r0neg512[p, c] = -float(_r0(b, g) * 512)
    return S_T, E_NEG, x_ramp, y_scal, r0neg512


def _build_nc():
    import concourse.bass as bass
    from concourse import bacc, mybir

    f32 = mybir.dt.float32
    i16 = mybir.dt.int16
    Alu = mybir.AluOpType

    nc = bacc.Bacc()
    img_p = nc.declare_dram_parameter("img", [CPC, N, N], f32, isOutput=False)
    cu_p = nc.declare_dram_parameter("c_u", [M, M], f32, isOutput=False)
    cv_p = nc.declare_dram_parameter("c_v", [M, M], f32, isOutput=False)
    st_p = nc.declare_dram_parameter("S_T", [M, N], f32, isOutput=False)
    en_p = nc.declare_dram_parameter("E_NEG", [M, M], f32, isOutput=False)
    xr_p = nc.declare_dram_parameter("x_ramp", [128, N], f32, isOutput=False)
    ys_p = nc.declare_dram_parameter("y_scal", [128, 4], f32, isOutput=False)
    r0_p = nc.declare_dram_parameter("r0neg512", [128, 4], f32, isOutput=False)
    out_p = nc.declare_dram_parameter("out", [CPC, N, N], f32, isOutput=True)

    idx_d = nc.dram_tensor("idx_dump", [4, N, N], i16)         # [map, y, x'=(s*32+q)]
    w_d2 = nc.dram_tensor("w_dump", [4, 8, 16, 64, N], f32)    # replicated weights

    st = ExitStack()
    sb = lambda name, shape, dt: st.enter_context(nc.sbuf_tensor(name, shape, dt))
    s_st = sb("s_st", [M, N], f32)
    s_en = sb("s_en", [M, M], f32)
    s_cu = sb("s_cu", [M, M], f32)
    s_cv = sb("s_cv", [M, M], f32)
    s_xr = sb("s_xr", [128, N], f32)
    s_ys = sb("s_ys", [128, 4], f32)
    s_r0 = sb("s_r0", [128, 4], f32)
    s_au = sb("s_au", [M, M], f32)
    s_av = sb("s_av", [M, M], f32)
    s_m1u = sb("s_m1u", [M, N], f32)
    s_m1v = sb("s_m1v", [M, N], f32)
    s_dxn = sb("s_dxn", [128, 4, N], f32)
    s_dyn = sb("s_dyn", [128, 4, N], f32)
    s_t = [sb(f"s_t{q}", [128, N], f32) for q in range(8)]
    s_tyf = sb("s_tyf", [128, N], f32)
    s_i32 = sb("s_i32", [128, N], mybir.dt.int32)
    s_wf = sb("s_wf", [128, 4, N], f32)
    s_if = sb("s_if", [128, 4, N], i16)
    s_idxw = sb("s_idxw", [128, 4, 2048], i16)
    s_band = sb("s_band", [128, BAND], f32)
    s_tap = sb("s_tap", [128, 4, SLICE], f32)
    s_ws = sb("s_ws", [128, 4, SLICE], f32)
    s_acc = sb("s_acc", [128, SLICE], f32)
    s_tmp = sb("s_tmp", [128, SLICE], f32)

    with (nc.Block() as block,
          nc.semaphore("dsem") as dsem,
          nc.semaphore("ldsem") as ldsem,
          nc.semaphore("asem") as asem,
          nc.semaphore("msem") as msem,
          nc.semaphore("xsem") as xsem,
          nc.semaphore("stg") as stg,
          nc.semaphore("dmp") as dmp,
          nc.semaphore("rdy") as rdy,
          nc.semaphore("iosem") as iosem,
          nc.semaphore("bsem") as bsem,
          nc.semaphore("gsem") as gsem,
          nc.semaphore("csem") as csem,
          nc.semaphore("wsem") as wsem,
          nc.semaphore("osem") as osem,
          nc.psum_tensor("ps_mu", [M, N], f32) as ps_mu,
          nc.psum_tensor("ps_mv", [M, N], f32) as ps_mv,
          nc.psum_tensor("ps_fa", [128, N], f32) as ps_fa,
          nc.psum_tensor("ps_fb", [128, N], f32) as ps_fb):

        @block.sync
        def _(eng):
            cnt = 0
            for dst, src in ((s_st, st_p), (s_en, en_p), (s_cu, cu_p), (s_cv, cv_p),
                             (s_xr, xr_p), (s_ys, ys_p), (s_r0, r0_p)):
                eng.dma_start(out=dst[:], in_=src[:]).then_inc(dsem, 16)
                cnt += 16
            eng.wait_ge(dsem, cnt)
            eng.nop().then_inc(ldsem, 1)
            # dump maps per chunk
            for j in range(4):
                eng.wait_ge(stg, j + 1)
                for m in range(4):
                    eng.dma_start(out=idx_d[m, 128 * j:128 * (j + 1), :],
                                  in_=s_if[:, m, :]).then_inc(dsem, 16)
                    cnt += 16
                for m in range(4):
                    for ss in range(16):
                        eng.dma_start(out=w_d2[m, 2 * j:2 * j + 2, ss],
                                      in_=s_wf[:, m, :]).then_inc(dsem, 16)
                        cnt += 16
                eng.wait_ge(dsem, cnt)
                eng.nop().then_inc(dmp, 1)
            # wrapped idx reload
            with nc.allow_non_contiguous_dma(reason="one-time 2B wrapped idx reload"):
                for m in range(4):
                    for b in range(8):
                        src_ap = idx_d[m, 64 * b:64 * b + 64, :].rearrange("rp (q s) -> s rp q", s=16)
                        dst_ap = s_idxw[16 * b:16 * b + 16, m, :].rearrange("p (rp q) -> p rp q", q=32)
                        eng.dma_start(out=dst_ap, in_=src_ap).then_inc(dsem, 16)
                        cnt += 16
            eng.wait_ge(dsem, cnt)
            eng.nop().then_inc(rdy, 1)
            # main loop DMA service
            for g in range(PASSES):
                if g > 0:
                    eng.wait_ge(gsem, g * SLICES)
                for b in range(8):
                    r0 = _r0(b, g)
                    eng.dma_start(out=s_band[16 * b:16 * b + CPC, :],
                                  in_=img_p[:, r0:r0 + BAND_ROWS, :].rearrange("c r x -> c (r x)")
                                  ).then_inc(bsem, 16)
                for t in range(SLICES):
                    sl = g * SLICES + t
                    eng.wait_ge(csem, sl)
                    for m in range(4):
                        lr = 32 * g + SLICE_ROWS * t
                        src = w_d2[m, :, :, lr:lr + SLICE_ROWS, :]
                        eng.dma_start(out=s_ws[:, m, :],
                                      in_=src.rearrange("b s r x -> (b s) (r x)")).then_inc(wsem, 16)
                    eng.wait_ge(csem, sl + 1)
                    y0 = 32 * g + SLICE_ROWS * t
                    for b in range(8):
                        eng.dma_start(out=out_p[:, 64 * b + y0:64 * b + y0 + SLICE_ROWS, :]
                                      .rearrange("c r x -> c (r x)"),
                                      in_=s_acc[16 * b:16 * b + CPC, :]).then_inc(osem, 16)
            eng.wait_ge(osem, 128 * NSL)

        @block.tensor
        def _(eng):
            eng.wait_ge(asem, 2)
            eng.matmul(ps_mu[:], s_au[:], s_st[:], start=True, stop=True).then_inc(msem, 1)
            eng.matmul(ps_mv[:], s_av[:], s_st[:], start=True, stop=True).then_inc(msem, 1)
            eng.wait_ge(xsem, 2)
            for j in range(4):
                if j > 0:
                    eng.wait_ge(xsem, 2 + 2 * j)
                eng.matmul(ps_fa[:], s_st[:, 128 * j:128 * (j + 1)], s_m1u[:],
                           start=True, stop=True).then_inc(msem, 1)
                eng.matmul(ps_fb[:], s_st[:, 128 * j:128 * (j + 1)], s_m1v[:],
                           start=True, stop=True).then_inc(msem, 1)

        @block.scalar
        def _(eng):
            eng.wait_ge(msem, 1)
            eng.copy(s_m1u[:], ps_mu[:])
            eng.maybe_drain_then_inc((xsem, 1))
            eng.wait_ge(msem, 2)
            eng.copy(s_m1v[:], ps_mv[:])
            eng.maybe_drain_then_inc((xsem, 1))
            for j in range(4):
                eng.wait_ge(msem, 3 + 2 * j)
                eng.copy(s_dxn[:, j, :], ps_fa[:])
                eng.maybe_drain_then_inc((xsem, 1))
                eng.wait_ge(msem, 4 + 2 * j)
                eng.copy(s_dyn[:, j, :], ps_fb[:])
                eng.maybe_drain_then_inc((xsem, 1))

        @block.vector
        def _(eng):
            eng.wait_ge(ldsem, 1)
            eng.tensor_tensor(s_au[:], s_cu[:], s_en[:], Alu.mult)
            eng.tensor_tensor(s_av[:], s_cv[:], s_en[:], Alu.mult)
            eng.maybe_drain_then_inc((asem, 2))
            t = s_t
            eng.wait_ge(iosem, 1)
            for j in range(4):
                eng.wait_ge(xsem, 4 + 2 * j)
                if j > 0:
                    eng.wait_ge(dmp, j)
                # helper: floor(src)->dst (exact under any int-convert rounding)
                def _floor(dst, src):
                    eng.tensor_copy(s_i32[:], src)
                    eng.tensor_copy(dst, s_i32[:])
                    eng.tensor_tensor(s_tmp[:, 0:N], dst, src, Alu.is_gt)
                    eng.tensor_tensor(dst, dst, s_tmp[:, 0:N], Alu.subtract)
                # y map and r0neg512 map from iota
                eng.tensor_scalar(t[7][:], s_tyf[:], float(128 * j), None, Alu.add)   # y
                eng.tensor_scalar(t[6][:], t[7][:], 1.0 / 32.0, None, Alu.mult)
                _floor(t[5][:], t[6][:])                                              # y//32
                eng.tensor_scalar(t[6][:], t[5][:], 32.0, None, Alu.mult)
                eng.tensor_scalar(t[6][:], t[6][:], -5.0, None, Alu.add)
                eng.tensor_scalar(t[6][:], t[6][:], 0.0, None, Alu.max)
                eng.tensor_scalar(t[6][:], t[6][:], float(N - BAND_ROWS), None, Alu.min)
                eng.tensor_scalar(t[6][:], t[6][:], -512.0, None, Alu.mult)           # r0neg512
                # yn = clip(y + (-dy)); xn = clip(x + (-dx))
                eng.tensor_tensor(t[1][:], s_dyn[:, j, :], t[7][:], Alu.add)
                eng.tensor_scalar(t[1][:], t[1][:], 0.0, None, Alu.max)
                eng.tensor_scalar(t[1][:], t[1][:], float(N - 1), None, Alu.min)
                eng.tensor_tensor(t[0][:], s_dxn[:, j, :], s_xr[:], Alu.add)
                eng.tensor_scalar(t[0][:], t[0][:], 0.0, None, Alu.max)
                eng.tensor_scalar(t[0][:], t[0][:], float(N - 1), None, Alu.min)
                _floor(t[3][:], t[0][:])                                     # xf
                eng.tensor_tensor(t[2][:], t[0][:], t[3][:], Alu.subtract)   # xv
                _floor(t[5][:], t[1][:])                                     # yf
                eng.tensor_tensor(t[4][:], t[1][:], t[5][:], Alu.subtract)   # yv
                eng.tensor_scalar(t[7][:], t[2][:], 0.0, None, Alu.is_gt)
                eng.tensor_tensor(t[7][:], t[3][:], t[7][:], Alu.add)        # xc
                eng.tensor_scalar(t[0][:], t[4][:], 0.0, None, Alu.is_gt)
                eng.tensor_tensor(t[0][:], t[5][:], t[0][:], Alu.add)        # yc
                eng.tensor_scalar(t[1][:], t[2][:], -1.0, None, Alu.mult)
                eng.tensor_scalar(t[1][:], t[1][:], 1.0, None, Alu.add)      # 1-xv
                eng.tensor_tensor(s_wf[:, 2, :], t[4][:], t[1][:], Alu.mult)
                eng.tensor_tensor(s_wf[:, 0, :], t[1][:], s_wf[:, 2, :], Alu.subtract)
                eng.tensor_tensor(s_wf[:, 3, :], t[4][:], t[2][:], Alu.mult)
                eng.tensor_tensor(s_wf[:, 1, :], t[2][:], s_wf[:, 3, :], Alu.subtract)
                eng.tensor_tensor(t[1][:], t[3][:], t[6][:], Alu.add)        # xf + r0n
                eng.tensor_tensor(t[2][:], t[7][:], t[6][:], Alu.add)        # xc + r0n
                eng.scalar_tensor_tensor(t[3][:], t[5][:], 512.0, t[1][:], Alu.mult, Alu.add)
                eng.scalar_tensor_tensor(t[4][:], t[5][:], 512.0, t[2][:], Alu.mult, Alu.add)
                eng.scalar_tensor_tensor(t[5][:], t[0][:], 512.0, t[1][:], Alu.mult, Alu.add)
                eng.scalar_tensor_tensor(t[1][:], t[0][:], 512.0, t[2][:], Alu.mult, Alu.add)
                for m, tt_ in enumerate((t[3], t[4], t[5], t[1])):
                    eng.tensor_copy(s_if[:, m, :], tt_[:])
                eng.maybe_drain_then_inc((stg, 1))
            # combine loop
            for sl in range(NSL):
                eng.wait_ge(gsem, sl + 1)
                eng.wait_ge(wsem, 64 * (sl + 1))
                if sl > 0:
                    eng.wait_ge(osem, 128 * sl)
                eng.tensor_tensor(s_acc[:], s_tap[:, 0, :], s_ws[:, 0, :], Alu.mult)
                for m in range(1, 4):
                    eng.tensor_tensor(s_tmp[:], s_tap[:, m, :], s_ws[:, m, :], Alu.mult)
                    eng.tensor_tensor(s_acc[:], s_acc[:], s_tmp[:], Alu.add)
                eng.maybe_drain_then_inc((csem, 1))

        @block.gpsimd
        def _(eng):
            eng.iota(s_tyf[:], [[0, N]], channel_multiplier=1,
                     allow_small_or_imprecise_dtypes=True)
            eng.maybe_drain_then_inc((iosem, 1))
            eng.wait_ge(rdy, 1)
            for g in range(PASSES):
                eng.wait_ge(bsem, 128 * (g + 1))
                for t_ in range(SLICES):
                    sl = g * SLICES + t_
                    if sl > 0:
                        eng.wait_ge(csem, sl)
                    ioff = (32 * g + SLICE_ROWS * t_) * 32
                    for m in range(4):
                        eng.ap_gather(
                            out_ap=s_tap[:, m, :], in_ap=s_band[:],
                            idxs_ap=s_idxw[:, m, ioff:ioff + SLICE // 16],
                            channels=128, num_elems=BAND, d=1, num_idxs=SLICE)
                    eng.maybe_drain_then_inc((gsem, 1))

    st.close()
    nc.compile()
    return nc


_COMPILED = None




class _CompiledBassKernel:
    """Compile once via PJRT (axon), run many times. Self-contained."""

    def __init__(self, nc, n_cores=8):
        import jax
        from jax.sharding import Mesh, PartitionSpec
        from jax.experimental.shard_map import shard_map
        from concourse import mybir
        from concourse.bass2jax import (install_neuronx_cc_hook, _bass_exec_p,
                                        partition_id_tensor)
        install_neuronx_cc_hook()
        self.n_cores = n_cores
        partition_name = nc.partition_id_tensor.name if nc.partition_id_tensor else None
        in_names, out_names, out_avals, zero_outs = [], [], [], []
        for alloc in nc.m.functions[0].allocations:
            if not isinstance(alloc, mybir.MemoryLocationSet):
                continue
            name = alloc.memorylocations[0].name
            if alloc.kind == "ExternalInput":
                if name != partition_name:
                    in_names.append(name)
            elif alloc.kind == "ExternalOutput":
                shape = tuple(alloc.tensor_shape)
                dtype = mybir.dt.np(alloc.dtype)
                out_names.append(name)
                out_avals.append(jax.core.ShapedArray(shape, dtype))
                zero_outs.append(np.zeros(shape, dtype))
        self.in_names, self.out_names = in_names, out_names
        self.out_avals, self.zero_outs = out_avals, zero_outs
        n_params = len(in_names)
        self.n_params = n_params
        all_in = list(in_names) + list(out_names)
        if partition_name is not None:
            all_in.append(partition_name)

        def _body(*args):
            operands = list(args)
            if partition_name is not None:
                operands.append(partition_id_tensor())
            outs = _bass_exec_p.bind(
                *operands, out_avals=tuple(out_avals), in_names=tuple(all_in),
                out_names=tuple(out_names), lowering_input_output_aliases=(),
                sim_require_finite=True, sim_require_nnan=True, nc=nc)
            return tuple(outs)

        donate = tuple(range(n_params, n_params + len(out_avals)))
        devices = jax.devices()[:n_cores]
        mesh = Mesh(np.asarray(devices), ("core",))
        in_specs = (PartitionSpec("core"),) * (n_params + len(out_avals))
        out_specs = (PartitionSpec("core"),) * len(out_names)
        self._jax = jax
        self._fn = jax.jit(
            shard_map(_body, mesh=mesh, in_specs=in_specs, out_specs=out_specs,
                      check_rep=False),
            donate_argnums=donate, keep_unused=True)

    def run(self, in_maps):
        n = self.n_cores
        per = [[np.asarray(m[k]) for k in self.in_names] for m in in_maps]
        cat = [np.concatenate([per[c][i] for c in range(n)], axis=0)
               for i in range(self.n_params)]
        zeros = [np.zeros((n * z.shape[0], *z.shape[1:]), z.dtype)
                 for z in self.zero_outs]
        outs = self._fn(*cat, *zeros)
        self._jax.block_until_ready(outs)
        return [{name: np.asarray(outs[i]).reshape(n, *self.out_avals[i].shape)[c]
                 for i, name in enumerate(self.out_names)}
                for c in range(n)]


def _get_compiled():
    global _COMPILED
    if _COMPILED is None:
        _COMPILED = _CompiledBassKernel(_build_nc(), NCORES)
    return _COMPILED


def kernel(img, c_u, c_v):
    img = np.asarray(img, dtype=np.float32)
    c_u = np.asarray(c_u, dtype=np.float32)
    c_v = np.asarray(c_v, dtype=np.float32)
    S_T, E_NEG, x_ramp, y_scal, r0neg512 = _constants()
    k = _get_compiled()
    B = img.shape[0]
    per = B // NCORES
    in_maps = []
    for core in range(NCORES):
        sl = img[core * per:(core + 1) * per].reshape(CPC, N, N)
        in_maps.append({
            "img": np.ascontiguousarray(sl), "c_u": c_u, "c_v": c_v,
            "S_T": S_T, "E_NEG": E_NEG, "x_ramp": x_ramp,
            "y_scal": y_scal, "r0neg512": r0neg512,
        })
    res = k.run(in_maps)
    return np.concatenate([r["out"].reshape(per, 3, N, N) for r in res], axis=0)


if __name__ == "__main__":
    import reference
    inputs = reference.setup_inputs()
    expected = np.asarray(reference.reference(**inputs))
    actual = kernel(**{kk: np.asarray(vv) for kk, vv in inputs.items()})
    err = np.linalg.norm(actual - expected) / np.linalg.norm(expected)
    print("Relative error:", err)



# revision 3
# speedup vs baseline: 1.5077x; 1.5077x over previous
"""Diffeomorphic image warp on Trainium2 (8 NeuronCores, batch-data-parallel).

out = bilinear_warp(img, dx, dy); dx/dy are smooth random fields from
100x100 mode coefficients via sin bases (input-independent bases baked
as constants). Per core: 12 channel-images (4 batches x 3 channels).

Host<->device traffic is the wall-clock bottleneck (axon tunnel), so img
is shipped as f16 and converted to f32 on device; out is produced as f16
and upcast on host. Output buffers and the constant tables live on device
across calls (no donation; the kernel writes every output element).

On-device pipeline per core:
  ACT:    f16 -> f32 img conversion (staged through s_band)
  PE:     dx/dy = S @ (c * E') @ S^T  (negated+scaled E' folded in)
  DVE:    index + weight maps (f32), exact floor/ceil via mod/is_gt
  DMA:    wrapped-index reorder via DRAM round-trip; weight replication
  GPSIMD: 4-tap ap_gather with block-shared wrapped int16 indices
  DVE:    bilinear combine (4 mult + 3 add), f16 store, DMA out
"""
import math
import sys
from contextlib import ExitStack

import numpy as np

sys.path.insert(0, "/opt/trn_rl_repo")

N = 512
M = 100
NCORES = 8
CPC = 12
BAND_ROWS = 44
BAND = BAND_ROWS * N
SLICE_ROWS = 2
SLICE = SLICE_ROWS * N
PASSES = 2
GROUP_ROWS = 32
SLICES = GROUP_ROWS // SLICE_ROWS
NSL = PASSES * SLICES
FLAT = CPC * N * N // 128          # 24576 f16 elems per partition
CCH = 6                            # conversion chunks
CW = FLAT // CCH                   # 4096 elems per chunk

STATIC_NAMES = ("S_T", "E_NEG", "x_ramp", "y_scal", "r0neg512")


def _r0(b, g):
    return min(max(64 * b + 32 * g - 5, 0), N - BAND_ROWS)


def _constants():
    log_cut = math.log(M + 1e-06)
    T1 = 1.0 / (math.pi * N ** 2 * log_cut)
    T2 = max(T1, 4.0 / (math.pi ** 3 * M ** 2 * log_cut))
    T = 0.5 * (T1 + T2)
    scale = math.sqrt(T) * N

    x = np.linspace(0.0, 1.0, N, dtype=np.float64)
    k = np.arange(1, M + 1, dtype=np.float64)
    i, j = np.meshgrid(k, k, indexing="ij")
    r = np.sqrt(i ** 2 + j ** 2)
    e = (r < M + 0.5).astype(np.float64) / r
    s = np.sin(np.pi * x[:, None] * k[None, :])
    S_T = np.ascontiguousarray(s.T).astype(np.float32)
    E_NEG = (-(e * scale)).astype(np.float32)

    x_ramp = np.tile(np.arange(N, dtype=np.float32), (128, 1))
    y_scal = np.zeros((128, 4), dtype=np.float32)
    r0neg512 = np.zeros((128, 4), dtype=np.float32)
    for p in range(128):
        for c in range(4):
            y = 128 * c + p
            y_scal[p, c] = y
            b, g = y // 64, (y // 32) % 2
            r0neg512[p, c] = -float(_r0(b, g) * 512)
    return S_T, E_NEG, x_ramp, y_scal, r0neg512


def _build_nc():
    import concourse.bass as bass
    from concourse import bacc, mybir

    f32 = mybir.dt.float32
    f16 = mybir.dt.float16
    i16 = mybir.dt.int16
    Alu = mybir.AluOpType

    nc = bacc.Bacc()
    img_p = nc.declare_dram_parameter("img", [128, FLAT], f16, isOutput=False)
    cu_p = nc.declare_dram_parameter("c_u", [M, M], f32, isOutput=False)
    cv_p = nc.declare_dram_parameter("c_v", [M, M], f32, isOutput=False)
    st_p = nc.declare_dram_parameter("S_T", [M, N], f32, isOutput=False)
    en_p = nc.declare_dram_parameter("E_NEG", [M, M], f32, isOutput=False)
    xr_p = nc.declare_dram_parameter("x_ramp", [128, N], f32, isOutput=False)
    ys_p = nc.declare_dram_parameter("y_scal", [128, 4], f32, isOutput=False)
    r0_p = nc.declare_dram_parameter("r0neg512", [128, 4], f32, isOutput=False)
    out_p = nc.declare_dram_parameter("out", [CPC, N, N], f16, isOutput=True)

    img32 = nc.dram_tensor("img32", [CPC, N, N], f32)
    idx_d = nc.dram_tensor("idx_dump", [4, N, N], i16)         # [map, y, x'=(s*32+q)]
    w_d2 = nc.dram_tensor("w_dump", [4, 8, 16, 64, N], f32)    # replicated weights

    img32f = img32[:].rearrange("c y x -> (c y x)").rearrange("(p e) -> p e", p=128)

    st = ExitStack()
    sb = lambda name, shape, dt: st.enter_context(nc.sbuf_tensor(name, shape, dt))
    s_st = sb("s_st", [M, N], f32)
    s_en = sb("s_en", [M, M], f32)
    s_cu = sb("s_cu", [M, M], f32)
    s_cv = sb("s_cv", [M, M], f32)
    s_xr = sb("s_xr", [128, N], f32)
    s_ys = sb("s_ys", [128, 4], f32)
    s_r0 = sb("s_r0", [128, 4], f32)
    s_au = sb("s_au", [M, M], f32)
    s_av = sb("s_av", [M, M], f32)
    s_m1u = sb("s_m1u", [M, N], f32)
    s_m1v = sb("s_m1v", [M, N], f32)
    s_dxn = sb("s_dxn", [128, 4, N], f32)
    s_dyn = sb("s_dyn", [128, 4, N], f32)
    s_t = [sb(f"s_t{q}", [128, N], f32) for q in range(8)]
    s_tyf = sb("s_tyf", [128, N], f32)
    s_i32 = sb("s_i32", [128, N], mybir.dt.int32)
    s_wf = sb("s_wf", [128, 4, N], f32)
    s_if = sb("s_if", [128, 4, N], i16)
    s_idxw = sb("s_idxw", [128, 4, 2048], i16)
    s_band = sb("s_band", [128, BAND], f32)
    s_tap = sb("s_tap", [128, 4, SLICE], f32)
    s_ws = sb("s_ws", [128, 4, SLICE], f32)
    s_acc = sb("s_acc", [128, SLICE], f32)
    s_tmp = sb("s_tmp", [128, SLICE], f32)
    s_o16 = sb("s_o16", [128, SLICE], f16)

    # f16->f32 conversion staging lives inside s_band (free until band loads):
    #   stage k%2: f16 view of f32 cols [2048*(k%2) : 2048*(k%2)+2048]
    #   res   k%2: f32 cols [4096 + 4096*(k%2) : ...+4096]
    stages = [s_band[:, 0:2048].bitcast(f16), s_band[:, 2048:4096].bitcast(f16)]
    reses = [s_band[:, 4096:8192], s_band[:, 8192:12288]]

    sem = lambda name: st.enter_context(nc.semaphore(name))
    (dsem, ldsem, asem, msem, xsem, stg, dmp, rdy, iosem, bsem, gsem,
     csem, wsem, osem, cisem, ccsem, cosem) = map(sem, (
        "dsem", "ldsem", "asem", "msem", "xsem", "stg", "dmp", "rdy",
        "iosem", "bsem", "gsem", "csem", "wsem", "osem", "cisem",
        "ccsem", "cosem"))
    ps = lambda name, shape: st.enter_context(nc.psum_tensor(name, shape, f32))
    ps_mu = ps("ps_mu", [M, N])
    ps_mv = ps("ps_mv", [M, N])
    ps_fa = ps("ps_fa", [128, N])
    ps_fb = ps("ps_fb", [128, N])

    with nc.Block() as block:

        @block.sync
        def _(eng):
            # f16 img conversion: stream chunks through s_band staging
            eng.dma_start(out=stages[0], in_=img_p[:, 0:CW]).then_inc(cisem, 16)
            eng.dma_start(out=stages[1], in_=img_p[:, CW:2 * CW]).then_inc(cisem, 16)
            cnt = 0
            for dst, src in ((s_st, st_p), (s_en, en_p), (s_cu, cu_p), (s_cv, cv_p),
                             (s_xr, xr_p), (s_ys, ys_p), (s_r0, r0_p)):
                eng.dma_start(out=dst[:], in_=src[:]).then_inc(dsem, 16)
                cnt += 16
            for k in range(CCH):
                eng.wait_ge(ccsem, k + 1)
                eng.dma_start(out=img32f[:, CW * k:CW * (k + 1)],
                              in_=reses[k % 2]).then_inc(cosem, 16)
                if k + 2 < CCH:
                    eng.dma_start(out=stages[k % 2],
                                  in_=img_p[:, CW * (k + 2):CW * (k + 3)]
                                  ).then_inc(cisem, 16)
            eng.wait_ge(dsem, cnt)
            eng.nop().then_inc(ldsem, 1)
            # dump maps per chunk
            for j in range(4):
                eng.wait_ge(stg, j + 1)
                for m in range(4):
                    eng.dma_start(out=idx_d[m, 128 * j:128 * (j + 1), :],
                                  in_=s_if[:, m, :]).then_inc(dsem, 16)
                    cnt += 16
                for m in range(4):
                    for ss in range(16):
                        eng.dma_start(out=w_d2[m, 2 * j:2 * j + 2, ss],
                                      in_=s_wf[:, m, :]).then_inc(dsem, 16)
                        cnt += 16
                eng.wait_ge(dsem, cnt)
                eng.nop().then_inc(dmp, 1)
            # wrapped idx reload
            with nc.allow_non_contiguous_dma(reason="one-time 2B wrapped idx reload"):
                for m in range(4):
                    for b in range(8):
                        src_ap = idx_d[m, 64 * b:64 * b + 64, :].rearrange("rp (q s) -> s rp q", s=16)
                        dst_ap = s_idxw[16 * b:16 * b + 16, m, :].rearrange("p (rp q) -> p rp q", q=32)
                        eng.dma_start(out=dst_ap, in_=src_ap).then_inc(dsem, 16)
                        cnt += 16
            eng.wait_ge(dsem, cnt)
            eng.nop().then_inc(rdy, 1)
            # main loop DMA service (band loads need the f32 img complete)
            eng.wait_ge(cosem, 16 * CCH)
            for g in range(PASSES):
                if g > 0:
                    eng.wait_ge(gsem, g * SLICES)
                for b in range(8):
                    r0 = _r0(b, g)
                    eng.dma_start(out=s_band[16 * b:16 * b + CPC, :],
                                  in_=img32[:, r0:r0 + BAND_ROWS, :].rearrange("c r x -> c (r x)")
                                  ).then_inc(bsem, 16)
                for t in range(SLICES):
                    sl = g * SLICES + t
                    eng.wait_ge(csem, sl)
                    for m in range(4):
                        lr = 32 * g + SLICE_ROWS * t
                        src = w_d2[m, :, :, lr:lr + SLICE_ROWS, :]
                        eng.dma_start(out=s_ws[:, m, :],
                                      in_=src.rearrange("b s r x -> (b s) (r x)")).then_inc(wsem, 16)
                    eng.wait_ge(csem, sl + 1)
                    y0 = 32 * g + SLICE_ROWS * t
                    for b in range(8):
                        eng.dma_start(out=out_p[:, 64 * b + y0:64 * b + y0 + SLICE_ROWS, :]
                                      .rearrange("c r x -> c (r x)"),
                                      in_=s_o16[16 * b:16 * b + CPC, :]).then_inc(osem, 16)
            eng.wait_ge(osem, 128 * NSL)

        @block.tensor
        def _(eng):
            eng.wait_ge(asem, 2)
            eng.matmul(ps_mu[:], s_au[:], s_st[:], start=True, stop=True).then_inc(msem, 1)
            eng.matmul(ps_mv[:], s_av[:], s_st[:], start=True, stop=True).then_inc(msem, 1)
            eng.wait_ge(xsem, 2)
            for j in range(4):
                if j > 0:
                    eng.wait_ge(xsem, 2 + 2 * j)
                eng.matmul(ps_fa[:], s_st[:, 128 * j:128 * (j + 1)], s_m1u[:],
                           start=True, stop=True).then_inc(msem, 1)
                eng.matmul(ps_fb[:], s_st[:, 128 * j:128 * (j + 1)], s_m1v[:],
                           start=True, stop=True).then_inc(msem, 1)

        @block.scalar
        def _(eng):
            # f16 -> f32 img conversion first (PE/DVE maps wait on these drains)
            for k in range(CCH):
                eng.wait_ge(cisem, 16 * (k + 1))
                if k >= 2:
                    eng.wait_ge(cosem, 16 * (k - 1))
                eng.copy(reses[k % 2], stages[k % 2])
                eng.maybe_drain_then_inc((ccsem, 1))
            eng.wait_ge(msem, 1)
            eng.copy(s_m1u[:], ps_mu[:])
            eng.maybe_drain_then_inc((xsem, 1))
            eng.wait_ge(msem, 2)
            eng.copy(s_m1v[:], ps_mv[:])
            eng.maybe_drain_then_inc((xsem, 1))
            for j in range(4):
                eng.wait_ge(msem, 3 + 2 * j)
                eng.copy(s_dxn[:, j, :], ps_fa[:])
                eng.maybe_drain_then_inc((xsem, 1))
                eng.wait_ge(msem, 4 + 2 * j)
                eng.copy(s_dyn[:, j, :], ps_fb[:])
                eng.maybe_drain_then_inc((xsem, 1))

        @block.vector
        def _(eng):
            eng.wait_ge(ldsem, 1)
            eng.tensor_tensor(s_au[:], s_cu[:], s_en[:], Alu.mult)
            eng.tensor_tensor(s_av[:], s_cv[:], s_en[:], Alu.mult)
            eng.maybe_drain_then_inc((asem, 2))
            t = s_t
            eng.wait_ge(iosem, 1)
            for j in range(4):
                eng.wait_ge(xsem, 4 + 2 * j)
                if j > 0:
                    eng.wait_ge(dmp, j)
                # helper: floor(src)->dst (exact under any int-convert rounding)
                def _floor(dst, src):
                    eng.tensor_copy(s_i32[:], src)
                    eng.tensor_copy(dst, s_i32[:])
                    eng.tensor_tensor(s_tmp[:, 0:N], dst, src, Alu.is_gt)
                    eng.tensor_tensor(dst, dst, s_tmp[:, 0:N], Alu.subtract)
                # y map and r0neg512 map from iota
                eng.tensor_scalar(t[7][:], s_tyf[:], float(128 * j), None, Alu.add)   # y
                eng.tensor_scalar(t[6][:], t[7][:], 1.0 / 32.0, None, Alu.mult)
                _floor(t[5][:], t[6][:])                                              # y//32
                eng.tensor_scalar(t[6][:], t[5][:], 32.0, None, Alu.mult)
                eng.tensor_scalar(t[6][:], t[6][:], -5.0, None, Alu.add)
                eng.tensor_scalar(t[6][:], t[6][:], 0.0, None, Alu.max)
                eng.tensor_scalar(t[6][:], t[6][:], float(N - BAND_ROWS), None, Alu.min)
                eng.tensor_scalar(t[6][:], t[6][:], -512.0, None, Alu.mult)           # r0neg512
                # yn = clip(y + (-dy)); xn = clip(x + (-dx))
                eng.tensor_tensor(t[1][:], s_dyn[:, j, :], t[7][:], Alu.add)
                eng.tensor_scalar(t[1][:], t[1][:], 0.0, None, Alu.max)
                eng.tensor_scalar(t[1][:], t[1][:], float(N - 1), None, Alu.min)
                eng.tensor_tensor(t[0][:], s_dxn[:, j, :], s_xr[:], Alu.add)
                eng.tensor_scalar(t[0][:], t[0][:], 0.0, None, Alu.max)
                eng.tensor_scalar(t[0][:], t[0][:], float(N - 1), None, Alu.min)
                _floor(t[3][:], t[0][:])                                     # xf
                eng.tensor_tensor(t[2][:], t[0][:], t[3][:], Alu.subtract)   # xv
                _floor(t[5][:], t[1][:])                                     # yf
                eng.tensor_tensor(t[4][:], t[1][:], t[5][:], Alu.subtract)   # yv
                eng.tensor_scalar(t[7][:], t[2][:], 0.0, None, Alu.is_gt)
                eng.tensor_tensor(t[7][:], t[3][:], t[7][:], Alu.add)        # xc
                eng.tensor_scalar(t[0][:], t[4][:], 0.0, None, Alu.is_gt)
                eng.tensor_tensor(t[0][:], t[5][:], t[0][:], Alu.add)        # yc
                eng.tensor_scalar(t[1][:], t[2][:], -1.0, None, Alu.mult)
                eng.tensor_scalar(t[1][:], t[1][:], 1.0, None, Alu.add)      # 1-xv
                eng.tensor_tensor(s_wf[:, 2, :], t[4][:], t[1][:], Alu.mult)
                eng.tensor_tensor(s_wf[:, 0, :], t[1][:], s_wf[:, 2, :], Alu.subtract)
                eng.tensor_tensor(s_wf[:, 3, :], t[4][:], t[2][:], Alu.mult)
                eng.tensor_tensor(s_wf[:, 1, :], t[2][:], s_wf[:, 3, :], Alu.subtract)
                eng.tensor_tensor(t[1][:], t[3][:], t[6][:], Alu.add)        # xf + r0n
                eng.tensor_tensor(t[2][:], t[7][:], t[6][:], Alu.add)        # xc + r0n
                eng.scalar_tensor_tensor(t[3][:], t[5][:], 512.0, t[1][:], Alu.mult, Alu.add)
                eng.scalar_tensor_tensor(t[4][:], t[5][:], 512.0, t[2][:], Alu.mult, Alu.add)
                eng.scalar_tensor_tensor(t[5][:], t[0][:], 512.0, t[1][:], Alu.mult, Alu.add)
                eng.scalar_tensor_tensor(t[1][:], t[0][:], 512.0, t[2][:], Alu.mult, Alu.add)
                for m, tt_ in enumerate((t[3], t[4], t[5], t[1])):
                    eng.tensor_copy(s_if[:, m, :], tt_[:])
                eng.maybe_drain_then_inc((stg, 1))
            # combine loop
            for sl in range(NSL):
                eng.wait_ge(gsem, sl + 1)
                eng.wait_ge(wsem, 64 * (sl + 1))
                if sl > 0:
                    eng.wait_ge(osem, 128 * sl)
                eng.tensor_tensor(s_acc[:], s_tap[:, 0, :], s_ws[:, 0, :], Alu.mult)
                for m in range(1, 3):
                    eng.tensor_tensor(s_tmp[:], s_tap[:, m, :], s_ws[:, m, :], Alu.mult)
                    eng.tensor_tensor(s_acc[:], s_acc[:], s_tmp[:], Alu.add)
                eng.tensor_tensor(s_tmp[:], s_tap[:, 3, :], s_ws[:, 3, :], Alu.mult)
                eng.tensor_tensor(s_o16[:], s_acc[:], s_tmp[:], Alu.add)
                eng.maybe_drain_then_inc((csem, 1))

        @block.gpsimd
        def _(eng):
            eng.iota(s_tyf[:], [[0, N]], channel_multiplier=1,
                     allow_small_or_imprecise_dtypes=True)
            eng.maybe_drain_then_inc((iosem, 1))
            eng.wait_ge(rdy, 1)
            for g in range(PASSES):
                eng.wait_ge(bsem, 128 * (g + 1))
                for t_ in range(SLICES):
                    sl = g * SLICES + t_
                    if sl > 0:
                        eng.wait_ge(csem, sl)
                    ioff = (32 * g + SLICE_ROWS * t_) * 32
                    for m in range(4):
                        eng.ap_gather(
                            out_ap=s_tap[:, m, :], in_ap=s_band[:],
                            idxs_ap=s_idxw[:, m, ioff:ioff + SLICE // 16],
                            channels=128, num_elems=BAND, d=1, num_idxs=SLICE)
                    eng.maybe_drain_then_inc((gsem, 1))

    st.close()
    nc.compile()
    return nc


_COMPILED = None


class _CompiledBassKernel:
    """Compile once via PJRT (axon), run many times. Self-contained.

    No donation: the kernel writes every element of `out`, so the zero
    output operands are dead and can live on device across runs. Static
    basis tables are also cached on device — per-run H2D is img (f16)
    plus the tiny mode-coefficient matrices.
    """

    def __init__(self, nc, n_cores=8):
        import jax
        from jax.sharding import Mesh, PartitionSpec
        from jax.experimental.shard_map import shard_map
        from concourse import mybir
        from concourse.bass2jax import (install_neuronx_cc_hook, _bass_exec_p,
                                        partition_id_tensor)
        install_neuronx_cc_hook()
        self.n_cores = n_cores
        partition_name = nc.partition_id_tensor.name if nc.partition_id_tensor else None
        in_names, out_names, out_avals, zero_outs = [], [], [], []
        for alloc in nc.m.functions[0].allocations:
            if not isinstance(alloc, mybir.MemoryLocationSet):
                continue
            name = alloc.memorylocations[0].name
            if alloc.kind == "ExternalInput":
                if name != partition_name:
                    in_names.append(name)
            elif alloc.kind == "ExternalOutput":
                shape = tuple(alloc.tensor_shape)
                dtype = mybir.dt.np(alloc.dtype)
                out_names.append(name)
                out_avals.append(jax.core.ShapedArray(shape, dtype))
                zero_outs.append(np.zeros(shape, dtype))
        self.in_names, self.out_names = in_names, out_names
        self.out_avals, self.zero_outs = out_avals, zero_outs
        n_params = len(in_names)
        self.n_params = n_params
        all_in = list(in_names) + list(out_names)
        if partition_name is not None:
            all_in.append(partition_name)

        def _body(*args):
            operands = list(args)
            if partition_name is not None:
                operands.append(partition_id_tensor())
            outs = _bass_exec_p.bind(
                *operands, out_avals=tuple(out_avals), in_names=tuple(all_in),
                out_names=tuple(out_names), lowering_input_output_aliases=(),
                sim_require_finite=True, sim_require_nnan=True, nc=nc)
            return tuple(outs)

        devices = jax.devices()[:n_cores]
        mesh = Mesh(np.asarray(devices), ("core",))
        in_specs = (PartitionSpec("core"),) * (n_params + len(out_avals))
        out_specs = (PartitionSpec("core"),) * len(out_names)
        self._jax = jax
        self._shard = jax.sharding.NamedSharding(mesh, PartitionSpec("core"))
        self._fn = jax.jit(
            shard_map(_body, mesh=mesh, in_specs=in_specs, out_specs=out_specs,
                      check_rep=False),
            keep_unused=True)
        self._static_dev = None
        self._zeros_dev = None

    def _ensure_resident(self, in_map0):
        """Upload static tables + zero output operands once."""
        if self._static_dev is not None:
            return
        jax = self._jax
        n = self.n_cores
        self._static_dev = {
            k: jax.device_put(
                np.concatenate([np.asarray(in_map0[k])] * n, axis=0), self._shard)
            for k in STATIC_NAMES}
        self._zeros_dev = [
            jax.device_put(np.zeros((n * z.shape[0], *z.shape[1:]), z.dtype),
                           self._shard)
            for z in self.zero_outs]
        jax.block_until_ready(list(self._static_dev.values()) + self._zeros_dev)

    def run(self, in_maps):
        n = self.n_cores
        self._ensure_resident(in_maps[0])
        args = []
        for name in self.in_names:
            if name in STATIC_NAMES:
                args.append(self._static_dev[name])
            else:
                args.append(np.concatenate(
                    [np.asarray(m[name]) for m in in_maps], axis=0))
        outs = self._fn(*args, *self._zeros_dev)
        self._jax.block_until_ready(outs)
        return [{name: np.asarray(outs[i]).reshape(n, *self.out_avals[i].shape)[c]
                 for i, name in enumerate(self.out_names)}
                for c in range(n)]


def _get_compiled():
    global _COMPILED
    if _COMPILED is None:
        _COMPILED = _CompiledBassKernel(_build_nc(), NCORES)
    return _COMPILED


def _make_in_maps(img, c_u, c_v, consts):
    S_T, E_NEG, x_ramp, y_scal, r0neg512 = consts
    c_u = np.asarray(c_u, dtype=np.float32)
    c_v = np.asarray(c_v, dtype=np.float32)
    B = img.shape[0]
    per = B // NCORES
    in_maps = []
    for core in range(NCORES):
        sl = np.ascontiguousarray(img[core * per:(core + 1) * per]).astype(
            np.float16).reshape(128, FLAT)
        in_maps.append({
            "img": sl, "c_u": c_u, "c_v": c_v,
            "S_T": S_T, "E_NEG": E_NEG, "x_ramp": x_ramp,
            "y_scal": y_scal, "r0neg512": r0neg512,
        })
    return in_maps


def kernel(img, c_u, c_v):
    img = np.asarray(img)
    k = _get_compiled()
    in_maps = _make_in_maps(img, c_u, c_v, _constants())
    res = k.run(in_maps)
    per = img.shape[0] // NCORES
    return np.concatenate(
        [r["out"].astype(np.float32).reshape(per, 3, N, N) for r in res], axis=0)


if __name__ == "__main__":
    import reference
    inputs = reference.setup_inputs()
    expected = np.asarray(reference.reference(**inputs))
    actual = kernel(**{kk: np.asarray(vv) for kk, vv in inputs.items()})
    err = np.linalg.norm(actual - expected) / np.linalg.norm(expected)
    print("Relative error:", err)


# revision 5
# speedup vs baseline: 2.0951x; 1.3896x over previous
"""Diffeomorphic image warp on Trainium2 (8 NeuronCores, batch-data-parallel).

out = bilinear_warp(img, dx, dy); dx/dy are smooth random fields from
100x100 mode coefficients via sin bases (input-independent bases baked
as constants). Per core: 12 channel-images (4 batches x 3 channels).

Host<->device traffic is the wall-clock bottleneck (axon tunnel), so img
is shipped as f16 and converted to f32 on device; out is produced as f16
and upcast on host. Output buffers and the constant tables live on device
across calls (no donation; the kernel writes every output element).

On-device pipeline per core:
  ACT:    f16 -> f32 img conversion (staged through s_band)
  PE:     dx/dy = S @ (c * E') @ S^T  (negated+scaled E' folded in)
  DVE:    index + weight maps (f32), exact floor/ceil via mod/is_gt
  DMA:    wrapped-index reorder via DRAM round-trip; weight replication
  GPSIMD: 4-tap ap_gather with block-shared wrapped int16 indices
  DVE:    bilinear combine (4 mult + 3 add), f16 store, DMA out
"""
import math
import sys
from contextlib import ExitStack

import numpy as np

sys.path.insert(0, "/opt/trn_rl_repo")

N = 512
M = 100
NCORES = 8
CPC = 12
BAND_ROWS = 44
BAND = BAND_ROWS * N
SLICE_ROWS = 2
SLICE = SLICE_ROWS * N
PASSES = 2
GROUP_ROWS = 32
SLICES = GROUP_ROWS // SLICE_ROWS
NSL = PASSES * SLICES
FLAT = CPC * N * N // 128          # 24576 f16 elems per partition
CCH = 6                            # conversion chunks
CW = FLAT // CCH                   # 4096 elems per chunk

STATIC_NAMES = ("S_T", "E_NEG", "x_ramp", "y_scal", "r0neg512")


def _r0(b, g):
    return min(max(64 * b + 32 * g - 5, 0), N - BAND_ROWS)


def _constants():
    log_cut = math.log(M + 1e-06)
    T1 = 1.0 / (math.pi * N ** 2 * log_cut)
    T2 = max(T1, 4.0 / (math.pi ** 3 * M ** 2 * log_cut))
    T = 0.5 * (T1 + T2)
    scale = math.sqrt(T) * N

    x = np.linspace(0.0, 1.0, N, dtype=np.float64)
    k = np.arange(1, M + 1, dtype=np.float64)
    i, j = np.meshgrid(k, k, indexing="ij")
    r = np.sqrt(i ** 2 + j ** 2)
    e = (r < M + 0.5).astype(np.float64) / r
    s = np.sin(np.pi * x[:, None] * k[None, :])
    S_T = np.ascontiguousarray(s.T).astype(np.float32)
    E_NEG = (-(e * scale)).astype(np.float32)

    x_ramp = np.tile(np.arange(N, dtype=np.float32), (128, 1))
    y_scal = np.zeros((128, 4), dtype=np.float32)
    r0neg512 = np.zeros((128, 4), dtype=np.float32)
    for p in range(128):
        for c in range(4):
            y = 128 * c + p
            y_scal[p, c] = y
            b, g = y // 64, (y // 32) % 2
            r0neg512[p, c] = -float(_r0(b, g) * 512)
    return S_T, E_NEG, x_ramp, y_scal, r0neg512


def _build_nc():
    import concourse.bass as bass
    from concourse import bacc, mybir

    f32 = mybir.dt.float32
    f16 = mybir.dt.float16
    i16 = mybir.dt.int16
    Alu = mybir.AluOpType

    nc = bacc.Bacc()
    img_p = nc.declare_dram_parameter("img", [128, FLAT], f16, isOutput=False)
    cu_p = nc.declare_dram_parameter("c_u", [M, M], f32, isOutput=False)
    cv_p = nc.declare_dram_parameter("c_v", [M, M], f32, isOutput=False)
    st_p = nc.declare_dram_parameter("S_T", [M, N], f32, isOutput=False)
    en_p = nc.declare_dram_parameter("E_NEG", [M, M], f32, isOutput=False)
    xr_p = nc.declare_dram_parameter("x_ramp", [128, N], f32, isOutput=False)
    ys_p = nc.declare_dram_parameter("y_scal", [128, 4], f32, isOutput=False)
    r0_p = nc.declare_dram_parameter("r0neg512", [128, 4], f32, isOutput=False)
    out_p = nc.declare_dram_parameter("out", [CPC, N, N], f16, isOutput=True)

    img32 = nc.dram_tensor("img32", [CPC, N, N], f32)
    idx_d = nc.dram_tensor("idx_dump", [4, N, N], i16)         # [map, y, x'=(s*32+q)]
    w_d2 = nc.dram_tensor("w_dump", [4, 8, 16, 64, N], f32)    # replicated weights

    img32f = img32[:].rearrange("c y x -> (c y x)").rearrange("(p e) -> p e", p=128)

    st = ExitStack()
    sb = lambda name, shape, dt: st.enter_context(nc.sbuf_tensor(name, shape, dt))
    s_st = sb("s_st", [M, N], f32)
    s_en = sb("s_en", [M, M], f32)
    s_cu = sb("s_cu", [M, M], f32)
    s_cv = sb("s_cv", [M, M], f32)
    s_xr = sb("s_xr", [128, N], f32)
    s_ys = sb("s_ys", [128, 4], f32)
    s_r0 = sb("s_r0", [128, 4], f32)
    s_au = sb("s_au", [M, M], f32)
    s_av = sb("s_av", [M, M], f32)
    s_m1u = sb("s_m1u", [M, N], f32)
    s_m1v = sb("s_m1v", [M, N], f32)
    s_dxn = sb("s_dxn", [128, 4, N], f32)
    s_dyn = sb("s_dyn", [128, 4, N], f32)
    s_t = [sb(f"s_t{q}", [128, N], f32) for q in range(8)]
    s_tyf = sb("s_tyf", [128, N], f32)
    s_i32 = sb("s_i32", [128, N], mybir.dt.int32)
    s_wf = sb("s_wf", [128, 4, N], f32)
    s_if = sb("s_if", [128, 4, N], i16)
    s_idxw = sb("s_idxw", [128, 4, 2048], i16)
    s_band = sb("s_band", [128, BAND], f32)
    s_tap = sb("s_tap", [128, 4, SLICE], f32)
    s_ws = sb("s_ws", [128, 4, SLICE], f32)
    s_acc = sb("s_acc", [128, SLICE], f32)
    s_tmp = sb("s_tmp", [128, SLICE], f32)
    s_o16 = sb("s_o16", [128, SLICE], f16)

    # f16->f32 conversion staging lives inside s_band (free until band loads):
    #   stage k%2: f16 view of f32 cols [2048*(k%2) : 2048*(k%2)+2048]
    #   res   k%2: f32 cols [4096 + 4096*(k%2) : ...+4096]
    stages = [s_band[:, 0:2048].bitcast(f16), s_band[:, 2048:4096].bitcast(f16)]
    reses = [s_band[:, 4096:8192], s_band[:, 8192:12288]]

    sem = lambda name: st.enter_context(nc.semaphore(name))
    (dsem, ldsem, asem, msem, xsem, stg, dmp, rdy, iosem, bsem, gsem,
     csem, wsem, osem, cisem, ccsem, cosem) = map(sem, (
        "dsem", "ldsem", "asem", "msem", "xsem", "stg", "dmp", "rdy",
        "iosem", "bsem", "gsem", "csem", "wsem", "osem", "cisem",
        "ccsem", "cosem"))
    ps = lambda name, shape: st.enter_context(nc.psum_tensor(name, shape, f32))
    ps_mu = ps("ps_mu", [M, N])
    ps_mv = ps("ps_mv", [M, N])
    ps_fa = ps("ps_fa", [128, N])
    ps_fb = ps("ps_fb", [128, N])

    with nc.Block() as block:

        @block.sync
        def _(eng):
            # f16 img conversion: stream chunks through s_band staging
            eng.dma_start(out=stages[0], in_=img_p[:, 0:CW]).then_inc(cisem, 16)
            eng.dma_start(out=stages[1], in_=img_p[:, CW:2 * CW]).then_inc(cisem, 16)
            cnt = 0
            for dst, src in ((s_st, st_p), (s_en, en_p), (s_cu, cu_p), (s_cv, cv_p),
                             (s_xr, xr_p), (s_ys, ys_p), (s_r0, r0_p)):
                eng.dma_start(out=dst[:], in_=src[:]).then_inc(dsem, 16)
                cnt += 16
            for k in range(CCH):
                eng.wait_ge(ccsem, k + 1)
                eng.dma_start(out=img32f[:, CW * k:CW * (k + 1)],
                              in_=reses[k % 2]).then_inc(cosem, 16)
                if k + 2 < CCH:
                    eng.dma_start(out=stages[k % 2],
                                  in_=img_p[:, CW * (k + 2):CW * (k + 3)]
                                  ).then_inc(cisem, 16)
            eng.wait_ge(dsem, cnt)
            eng.nop().then_inc(ldsem, 1)
            # dump maps per chunk
            for j in range(4):
                eng.wait_ge(stg, j + 1)
                for m in range(4):
                    eng.dma_start(out=idx_d[m, 128 * j:128 * (j + 1), :],
                                  in_=s_if[:, m, :]).then_inc(dsem, 16)
                    cnt += 16
                for m in range(4):
                    for ss in range(16):
                        eng.dma_start(out=w_d2[m, 2 * j:2 * j + 2, ss],
                                      in_=s_wf[:, m, :]).then_inc(dsem, 16)
                        cnt += 16
                eng.wait_ge(dsem, cnt)
                eng.nop().then_inc(dmp, 1)
            # wrapped idx reload
            with nc.allow_non_contiguous_dma(reason="one-time 2B wrapped idx reload"):
                for m in range(4):
                    for b in range(8):
                        src_ap = idx_d[m, 64 * b:64 * b + 64, :].rearrange("rp (q s) -> s rp q", s=16)
                        dst_ap = s_idxw[16 * b:16 * b + 16, m, :].rearrange("p (rp q) -> p rp q", q=32)
                        eng.dma_start(out=dst_ap, in_=src_ap).then_inc(dsem, 16)
                        cnt += 16
            eng.wait_ge(dsem, cnt)
            eng.nop().then_inc(rdy, 1)
            # main loop DMA service (band loads need the f32 img complete)
            eng.wait_ge(cosem, 16 * CCH)
            for g in range(PASSES):
                if g > 0:
                    eng.wait_ge(gsem, g * SLICES)
                for b in range(8):
                    r0 = _r0(b, g)
                    eng.dma_start(out=s_band[16 * b:16 * b + CPC, :],
                                  in_=img32[:, r0:r0 + BAND_ROWS, :].rearrange("c r x -> c (r x)")
                                  ).then_inc(bsem, 16)
                for t in range(SLICES):
                    sl = g * SLICES + t
                    eng.wait_ge(csem, sl)
                    for m in range(4):
                        lr = 32 * g + SLICE_ROWS * t
                        src = w_d2[m, :, :, lr:lr + SLICE_ROWS, :]
                        eng.dma_start(out=s_ws[:, m, :],
                                      in_=src.rearrange("b s r x -> (b s) (r x)")).then_inc(wsem, 16)
                    eng.wait_ge(csem, sl + 1)
                    y0 = 32 * g + SLICE_ROWS * t
                    for b in range(8):
                        eng.dma_start(out=out_p[:, 64 * b + y0:64 * b + y0 + SLICE_ROWS, :]
                                      .rearrange("c r x -> c (r x)"),
                                      in_=s_o16[16 * b:16 * b + CPC, :]).then_inc(osem, 16)
            eng.wait_ge(osem, 128 * NSL)

        @block.tensor
        def _(eng):
            eng.wait_ge(asem, 2)
            eng.matmul(ps_mu[:], s_au[:], s_st[:], start=True, stop=True).then_inc(msem, 1)
            eng.matmul(ps_mv[:], s_av[:], s_st[:], start=True, stop=True).then_inc(msem, 1)
            eng.wait_ge(xsem, 2)
            for j in range(4):
                if j > 0:
                    eng.wait_ge(xsem, 2 + 2 * j)
                eng.matmul(ps_fa[:], s_st[:, 128 * j:128 * (j + 1)], s_m1u[:],
                           start=True, stop=True).then_inc(msem, 1)
                eng.matmul(ps_fb[:], s_st[:, 128 * j:128 * (j + 1)], s_m1v[:],
                           start=True, stop=True).then_inc(msem, 1)

        @block.scalar
        def _(eng):
            # f16 -> f32 img conversion first (PE/DVE maps wait on these drains)
            for k in range(CCH):
                eng.wait_ge(cisem, 16 * (k + 1))
                if k >= 2:
                    eng.wait_ge(cosem, 16 * (k - 1))
                eng.copy(reses[k % 2], stages[k % 2])
                eng.maybe_drain_then_inc((ccsem, 1))
            eng.wait_ge(msem, 1)
            eng.copy(s_m1u[:], ps_mu[:])
            eng.maybe_drain_then_inc((xsem, 1))
            eng.wait_ge(msem, 2)
            eng.copy(s_m1v[:], ps_mv[:])
            eng.maybe_drain_then_inc((xsem, 1))
            for j in range(4):
                eng.wait_ge(msem, 3 + 2 * j)
                eng.copy(s_dxn[:, j, :], ps_fa[:])
                eng.maybe_drain_then_inc((xsem, 1))
                eng.wait_ge(msem, 4 + 2 * j)
                eng.copy(s_dyn[:, j, :], ps_fb[:])
                eng.maybe_drain_then_inc((xsem, 1))

        @block.vector
        def _(eng):
            eng.wait_ge(ldsem, 1)
            eng.tensor_tensor(s_au[:], s_cu[:], s_en[:], Alu.mult)
            eng.tensor_tensor(s_av[:], s_cv[:], s_en[:], Alu.mult)
            eng.maybe_drain_then_inc((asem, 2))
            t = s_t
            eng.wait_ge(iosem, 1)
            for j in range(4):
                eng.wait_ge(xsem, 4 + 2 * j)
                if j > 0:
                    eng.wait_ge(dmp, j)
                # helper: floor(src)->dst (exact under any int-convert rounding)
                def _floor(dst, src):
                    eng.tensor_copy(s_i32[:], src)
                    eng.tensor_copy(dst, s_i32[:])
                    eng.tensor_tensor(s_tmp[:, 0:N], dst, src, Alu.is_gt)
                    eng.tensor_tensor(dst, dst, s_tmp[:, 0:N], Alu.subtract)
                # y map and r0neg512 map from iota
                eng.tensor_scalar(t[7][:], s_tyf[:], float(128 * j), None, Alu.add)   # y
                eng.tensor_scalar(t[6][:], t[7][:], 1.0 / 32.0, None, Alu.mult)
                _floor(t[5][:], t[6][:])                                              # y//32
                eng.tensor_scalar(t[6][:], t[5][:], 32.0, None, Alu.mult)
                eng.tensor_scalar(t[6][:], t[6][:], -5.0, None, Alu.add)
                eng.tensor_scalar(t[6][:], t[6][:], 0.0, None, Alu.max)
                eng.tensor_scalar(t[6][:], t[6][:], float(N - BAND_ROWS), None, Alu.min)
                eng.tensor_scalar(t[6][:], t[6][:], -512.0, None, Alu.mult)           # r0neg512
                # yn = clip(y + (-dy)); xn = clip(x + (-dx))
                eng.tensor_tensor(t[1][:], s_dyn[:, j, :], t[7][:], Alu.add)
                eng.tensor_scalar(t[1][:], t[1][:], 0.0, None, Alu.max)
                eng.tensor_scalar(t[1][:], t[1][:], float(N - 1), None, Alu.min)
                eng.tensor_tensor(t[0][:], s_dxn[:, j, :], s_xr[:], Alu.add)
                eng.tensor_scalar(t[0][:], t[0][:], 0.0, None, Alu.max)
                eng.tensor_scalar(t[0][:], t[0][:], float(N - 1), None, Alu.min)
                _floor(t[3][:], t[0][:])                                     # xf
                eng.tensor_tensor(t[2][:], t[0][:], t[3][:], Alu.subtract)   # xv
                _floor(t[5][:], t[1][:])                                     # yf
                eng.tensor_tensor(t[4][:], t[1][:], t[5][:], Alu.subtract)   # yv
                eng.tensor_scalar(t[7][:], t[2][:], 0.0, None, Alu.is_gt)
                eng.tensor_tensor(t[7][:], t[3][:], t[7][:], Alu.add)        # xc
                eng.tensor_scalar(t[0][:], t[4][:], 0.0, None, Alu.is_gt)
                eng.tensor_tensor(t[0][:], t[5][:], t[0][:], Alu.add)        # yc
                eng.tensor_scalar(t[1][:], t[2][:], -1.0, None, Alu.mult)
                eng.tensor_scalar(t[1][:], t[1][:], 1.0, None, Alu.add)      # 1-xv
                eng.tensor_tensor(s_wf[:, 2, :], t[4][:], t[1][:], Alu.mult)
                eng.tensor_tensor(s_wf[:, 0, :], t[1][:], s_wf[:, 2, :], Alu.subtract)
                eng.tensor_tensor(s_wf[:, 3, :], t[4][:], t[2][:], Alu.mult)
                eng.tensor_tensor(s_wf[:, 1, :], t[2][:], s_wf[:, 3, :], Alu.subtract)
                eng.tensor_tensor(t[1][:], t[3][:], t[6][:], Alu.add)        # xf + r0n
                eng.tensor_tensor(t[2][:], t[7][:], t[6][:], Alu.add)        # xc + r0n
                eng.scalar_tensor_tensor(t[3][:], t[5][:], 512.0, t[1][:], Alu.mult, Alu.add)
                eng.scalar_tensor_tensor(t[4][:], t[5][:], 512.0, t[2][:], Alu.mult, Alu.add)
                eng.scalar_tensor_tensor(t[5][:], t[0][:], 512.0, t[1][:], Alu.mult, Alu.add)
                eng.scalar_tensor_tensor(t[1][:], t[0][:], 512.0, t[2][:], Alu.mult, Alu.add)
                for m, tt_ in enumerate((t[3], t[4], t[5], t[1])):
                    eng.tensor_copy(s_if[:, m, :], tt_[:])
                eng.maybe_drain_then_inc((stg, 1))
            # combine loop
            for sl in range(NSL):
                eng.wait_ge(gsem, sl + 1)
                eng.wait_ge(wsem, 64 * (sl + 1))
                if sl > 0:
                    eng.wait_ge(osem, 128 * sl)
                eng.tensor_tensor(s_acc[:], s_tap[:, 0, :], s_ws[:, 0, :], Alu.mult)
                for m in range(1, 3):
                    eng.tensor_tensor(s_tmp[:], s_tap[:, m, :], s_ws[:, m, :], Alu.mult)
                    eng.tensor_tensor(s_acc[:], s_acc[:], s_tmp[:], Alu.add)
                eng.tensor_tensor(s_tmp[:], s_tap[:, 3, :], s_ws[:, 3, :], Alu.mult)
                eng.tensor_tensor(s_o16[:], s_acc[:], s_tmp[:], Alu.add)
                eng.maybe_drain_then_inc((csem, 1))

        @block.gpsimd
        def _(eng):
            eng.iota(s_tyf[:], [[0, N]], channel_multiplier=1,
                     allow_small_or_imprecise_dtypes=True)
            eng.maybe_drain_then_inc((iosem, 1))
            eng.wait_ge(rdy, 1)
            for g in range(PASSES):
                eng.wait_ge(bsem, 128 * (g + 1))
                for t_ in range(SLICES):
                    sl = g * SLICES + t_
                    if sl > 0:
                        eng.wait_ge(csem, sl)
                    ioff = (32 * g + SLICE_ROWS * t_) * 32
                    for m in range(4):
                        eng.ap_gather(
                            out_ap=s_tap[:, m, :], in_ap=s_band[:],
                            idxs_ap=s_idxw[:, m, ioff:ioff + SLICE // 16],
                            channels=128, num_elems=BAND, d=1, num_idxs=SLICE)
                    eng.maybe_drain_then_inc((gsem, 1))

    st.close()
    nc.compile()
    return nc


_COMPILED = None


class _CompiledBassKernel:
    """Compile once via PJRT (axon), run many times. Self-contained.

    No donation: the kernel writes every element of `out`, so the zero
    output operands are dead and can live on device across runs. Static
    basis tables are also cached on device — per-run H2D is img (f16)
    plus the tiny mode-coefficient matrices.
    """

    def __init__(self, nc, n_cores=8):
        import jax
        from jax.sharding import Mesh, PartitionSpec
        from jax.experimental.shard_map import shard_map
        from concourse import mybir
        from concourse.bass2jax import (install_neuronx_cc_hook, _bass_exec_p,
                                        partition_id_tensor)
        install_neuronx_cc_hook()
        self.n_cores = n_cores
        partition_name = nc.partition_id_tensor.name if nc.partition_id_tensor else None
        in_names, out_names, out_avals, zero_outs = [], [], [], []
        for alloc in nc.m.functions[0].allocations:
            if not isinstance(alloc, mybir.MemoryLocationSet):
                continue
            name = alloc.memorylocations[0].name
            if alloc.kind == "ExternalInput":
                if name != partition_name:
                    in_names.append(name)
            elif alloc.kind == "ExternalOutput":
                shape = tuple(alloc.tensor_shape)
                dtype = mybir.dt.np(alloc.dtype)
                out_names.append(name)
                out_avals.append(jax.core.ShapedArray(shape, dtype))
                zero_outs.append(np.zeros(shape, dtype))
        self.in_names, self.out_names = in_names, out_names
        self.out_avals, self.zero_outs = out_avals, zero_outs
        n_params = len(in_names)
        self.n_params = n_params
        all_in = list(in_names) + list(out_names)
        if partition_name is not None:
            all_in.append(partition_name)

        def _body(*args):
            operands = list(args)
            if partition_name is not None:
                operands.append(partition_id_tensor())
            outs = _bass_exec_p.bind(
                *operands, out_avals=tuple(out_avals), in_names=tuple(all_in),
                out_names=tuple(out_names), lowering_input_output_aliases=(),
                sim_require_finite=True, sim_require_nnan=True, nc=nc)
            return tuple(outs)

        devices = jax.devices()[:n_cores]
        mesh = Mesh(np.asarray(devices), ("core",))
        in_specs = (PartitionSpec("core"),) * (n_params + len(out_avals))
        out_specs = (PartitionSpec("core"),) * len(out_names)
        self._jax = jax
        self._shard = jax.sharding.NamedSharding(mesh, PartitionSpec("core"))
        self._fn = jax.jit(
            shard_map(_body, mesh=mesh, in_specs=in_specs, out_specs=out_specs,
                      check_rep=False),
            keep_unused=True)
        self._static_dev = None
        self._zeros_dev = None

    def _ensure_resident(self, in_map0):
        """Upload static tables + zero output operands once."""
        if self._static_dev is not None:
            return
        jax = self._jax
        n = self.n_cores
        self._static_dev = {
            k: jax.device_put(
                np.concatenate([np.asarray(in_map0[k])] * n, axis=0), self._shard)
            for k in STATIC_NAMES}
        self._zeros_dev = [
            jax.device_put(np.zeros((n * z.shape[0], *z.shape[1:]), z.dtype),
                           self._shard)
            for z in self.zero_outs]
        jax.block_until_ready(list(self._static_dev.values()) + self._zeros_dev)

    def run(self, in_maps):
        n = self.n_cores
        self._ensure_resident(in_maps[0])
        args = []
        for name in self.in_names:
            if name in STATIC_NAMES:
                args.append(self._static_dev[name])
            else:
                args.append(np.concatenate(
                    [np.asarray(m[name]) for m in in_maps], axis=0))
        outs = self._fn(*args, *self._zeros_dev)
        self._jax.block_until_ready(outs)
        return [{name: np.asarray(outs[i]).reshape(n, *self.out_avals[i].shape)[c]
                 for i, name in enumerate(self.out_names)}
                for c in range(n)]

    def run_flat(self, img_flat, c_u_cat, c_v_cat, statics):
        """img_flat: [8*128, FLAT] f16; returns global out array [8*CPC,N,N] f16."""
        self._ensure_resident(statics)
        args = []
        for name in self.in_names:
            if name == "img":
                args.append(img_flat)
            elif name == "c_u":
                args.append(c_u_cat)
            elif name == "c_v":
                args.append(c_v_cat)
            else:
                args.append(self._static_dev[name])
        outs = self._fn(*args, *self._zeros_dev)
        self._jax.block_until_ready(outs)
        return np.asarray(outs[0])


def _get_compiled():
    global _COMPILED
    if _COMPILED is None:
        _COMPILED = _CompiledBassKernel(_build_nc(), NCORES)
    return _COMPILED


def _make_in_maps(img, c_u, c_v, consts):
    S_T, E_NEG, x_ramp, y_scal, r0neg512 = consts
    c_u = np.asarray(c_u, dtype=np.float32)
    c_v = np.asarray(c_v, dtype=np.float32)
    B = img.shape[0]
    per = B // NCORES
    in_maps = []
    for core in range(NCORES):
        sl = np.ascontiguousarray(img[core * per:(core + 1) * per]).astype(
            np.float16).reshape(128, FLAT)
        in_maps.append({
            "img": sl, "c_u": c_u, "c_v": c_v,
            "S_T": S_T, "E_NEG": E_NEG, "x_ramp": x_ramp,
            "y_scal": y_scal, "r0neg512": r0neg512,
        })
    return in_maps


def kernel(img, c_u, c_v):
    from concurrent.futures import ThreadPoolExecutor
    img = np.ascontiguousarray(np.asarray(img), dtype=np.float32)
    k = _get_compiled()
    statics = dict(zip(STATIC_NAMES, _constants()))
    B = img.shape[0]
    src = img.reshape(NCORES, 128, FLAT)
    imgf = np.empty((NCORES * 128, FLAT), np.float16)

    def _conv(c):
        np.copyto(imgf[c * 128:(c + 1) * 128], src[c], casting="same_kind")

    with ThreadPoolExecutor(NCORES) as ex:
        list(ex.map(_conv, range(NCORES)))
    c_u_cat = np.tile(np.asarray(c_u, dtype=np.float32), (NCORES, 1))
    c_v_cat = np.tile(np.asarray(c_v, dtype=np.float32), (NCORES, 1))
    full = k.run_flat(imgf, c_u_cat, c_v_cat, statics)   # [8*CPC, N, N] f16
    out = np.empty(full.shape, np.float32)

    def _up(c):
        np.copyto(out[c * CPC:(c + 1) * CPC], full[c * CPC:(c + 1) * CPC],
                  casting="same_kind")

    with ThreadPoolExecutor(NCORES) as ex:
        list(ex.map(_up, range(NCORES)))
    return out.reshape(B, 3, N, N)


if __name__ == "__main__":
    import reference
    inputs = reference.setup_inputs()
    expected = np.asarray(reference.reference(**inputs))
    actual = kernel(**{kk: np.asarray(vv) for kk, vv in inputs.items()})
    err = np.linalg.norm(actual - expected) / np.linalg.norm(expected)
    print("Relative error:", err)
